# revision 1
# baseline (speedup 1.0000x reference)
"""Bahdanau attention on 8 Trainium2 NeuronCores (Bass/Tile).

Problem:  S=2048, B=32, D=1024, E2=1024
  ws  = dec @ Wb.T                       [B, D]
  WH  = enc @ Wc.T                       [S, B, D]
  sc  = tanh(WH + ws) . Wa               [S, B]
  at  = softmax(sc, axis=0)              [S, B]
  out = einsum('sb,sbe->be', at, enc)[None]   [1, B, 2E]

Sharding: data-parallel over batch B across 8 cores (4 batches/core);
Wb/Wc/Wa replicated. Softmax axis (S) stays core-local.

enc is staged host-side in two layouts per core shard: natural [S, BL, E2]
(context contraction over s needs s on partitions) and transposed
[E2, BL, S] (the Wc matmul contracts over e, which must sit on partitions).
Both are DMA'd with per-partition-contiguous rows; the fp32->fp32r (TF32)
cast happens in the DMA so every matmul runs at 1 col/cycle.

Per-core dataflow:
  - WH^T [d-chunk=128p, s'] = (WcT chunk).T @ encT tile, accumulated over e
  - ACT: tanh(WH + wsT[d,b]) fused via per-partition bias
  - score row [1, s'] = sum_d WaChunk.T @ tanh_chunk  (PE)
  - exp on ACT (no max subtraction: |score| <= sum|Wa| ~ 26, exp fits fp32
    comfortably and softmax is shift-invariant), Z via activation accum_out
  - exp row -> column via a K=32 matmul against e0 (rows 1-31 zeroed)
  - context [1, e] += expCol.T @ enc_nat on PE across all s-tiles
    (unnormalized), scaled by 1/Z once per batch at the end.

Engines run their instruction streams in order, so emission order doubles as
a schedule: tile(0,0)'s enc DMAs are emitted before the weight DMAs to cover
DMA latency at kernel start.
"""

import numpy as np

S, B, D, E2 = 2048, 32, 1024, 1024
NCORES = 8
BL = B // NCORES          # batches per core
ST = 512                  # s-tile size
NST = S // ST             # s-tiles per batch
NSUB = ST // 128          # 128-row subtiles per s-tile
EC = E2 // 128            # e chunks
DC = D // 128             # d chunks

_CACHE = {}


def _build_nc():
    import concourse.bacc as bacc
    import concourse.tile as tile
    from concourse import mybir
    from concourse.masks import make_identity

    f32 = mybir.dt.float32
    f32r = mybir.dt.float32r
    f16 = mybir.dt.float16
    TANH = mybir.ActivationFunctionType.Tanh
    EXP = mybir.ActivationFunctionType.Exp
    X = mybir.AxisListType.X

    nc = bacc.Bacc()
    enc = nc.declare_dram_parameter("enc", [S, BL, E2], f32, isOutput=False)
    enct_h = nc.declare_dram_parameter("enct", [E2, BL, S], f32, isOutput=False)
    dect = nc.declare_dram_parameter("dect", [D, BL], f32, isOutput=False)
    wbt = nc.declare_dram_parameter("wbt", [D, D], f32, isOutput=False)    # Wb.T [d, d2]
    wct = nc.declare_dram_parameter("wct", [E2, D], f32, isOutput=False)   # Wc.T [e, d]
    wa2 = nc.declare_dram_parameter("wa2", [128, DC], f32, isOutput=False) # Wa chunks as cols
    outp = nc.declare_dram_parameter("out", [BL, E2], f32, isOutput=True)

    with tile.TileContext(nc) as tc:
        with (
            tc.tile_pool(name="const", bufs=1) as const_pool,
            tc.tile_pool(name="wbtp", bufs=1) as wbt_pool,
            tc.tile_pool(name="encn", bufs=3) as encn_pool,
            tc.tile_pool(name="enct", bufs=2) as enct_pool,
            tc.tile_pool(name="tanhp", bufs=4) as tanh_pool,
            tc.tile_pool(name="rows", bufs=2) as row_pool,
            tc.tile_pool(name="wh_ps", bufs=4, space="PSUM") as wh_ps,
            tc.tile_pool(name="sc_ps", bufs=2, space="PSUM") as sc_ps,
            tc.tile_pool(name="ctx_ps", bufs=2, space="PSUM") as ctx_ps,
        ):
            ex_ps = wh_ps  # columnize rides the wh rotation (same tag below)

            # ---- identity (fp32 via gpsimd) + fp16 e0 for the columnize ----
            id32 = const_pool.tile([128, 128], f32)
            make_identity(nc, id32)
            e0f16 = const_pool.tile([32, 1], f16)
            nc.vector.tensor_copy(out=e0f16, in_=id32[0:32, 0:1])

            # ---- emission helpers (order == per-engine schedule) ----
            # One fused dma_start per tile load: SWDGE descriptor generation
            # costs ~0.6us per dma_start on the GpSimd Q7, so batch the
            # subtiles into a single 2 MB transfer with a 3-D access pattern.
            def load_subs(bj, st, pfx=""):
                s0 = st * ST
                sub_big = encn_pool.tile([128, NSUB, E2], f32r, tag="sub",
                                         name=f"sub{pfx}")
                nc.gpsimd.dma_start(
                    out=sub_big,
                    in_=enc[s0:s0 + ST, bj, :].rearrange("(j p) e -> p j e", p=128),
                )
                return [sub_big[:, j, :] for j in range(NSUB)]

            def load_enct(bj, st, pfx=""):
                s0 = st * ST
                enct = []
                for ecn in range(EC):
                    et = enct_pool.tile([128, ST], f32r, tag="et", bufs=16,
                                        name=f"et{pfx}_{ecn}")
                    nc.gpsimd.dma_start(
                        out=et,
                        in_=enct_h[ecn * 128:(ecn + 1) * 128, bj, s0:s0 + ST],
                    )
                    enct.append(et)
                return enct

            # DMA queue order tuned for startup, strictly by first PE need:
            # ws inputs (dect tiny, then wbt chunk-paced), then the WH-path
            # tile(0,0) load, then Wc^T, then the ctx-path tile(0,0) load.
            dect_big = const_pool.tile([128, DC, BL], f32r)
            nc.gpsimd.dma_start(
                out=dect_big, in_=dect[:, :].rearrange("(c p) b -> p c b", p=128)
            )
            dect_sb = [dect_big[:, dk, :] for dk in range(DC)]
            wbt_sb = []
            for dk in range(DC):
                t = wbt_pool.tile([128, D], f32r, tag="wbt_sb", bufs=DC, name=f"wbt{dk}")
                nc.gpsimd.dma_start(out=t, in_=wbt[dk * 128:(dk + 1) * 128, :])
                wbt_sb.append(t)
            enct_cache = {(0, 0): load_enct(0, 0, pfx="00")}
            wct_sb = []
            for ecn in range(EC):
                t = const_pool.tile([128, D], f32r, tag="wct_sb", bufs=EC, name=f"wct{ecn}")
                nc.gpsimd.dma_start(out=t, in_=wct[ecn * 128:(ecn + 1) * 128, :])
                wct_sb.append(t)
            subs_cache = {(0, 0): load_subs(0, 0, pfx="00")}
            wa_sb = const_pool.tile([128, DC], f32r)
            nc.gpsimd.dma_start(out=wa_sb, in_=wa2[:, :])

            # ---- ws = dec @ Wb.T -> wsT [d2-chunk, b] for the tanh bias ----
            # ws natural [BL, d2]: lhsT = dectChunk [dk, BL], rhs = wbtChunk
            ws_sb = const_pool.tile([BL, D], f32r)
            ws_psum = [wh_ps.tile([BL, 512], f32, tag="wh", name=f"ws_psum{eh}")
                       for eh in range(2)]
            for dk in range(DC):
                for eh in range(2):
                    nc.tensor.matmul(
                        ws_psum[eh], dect_sb[dk], wbt_sb[dk][:, eh * 512:(eh + 1) * 512],
                        start=(dk == 0), stop=(dk == DC - 1),
                    )
            for eh in range(2):
                nc.scalar.copy(out=ws_sb[:, eh * 512:(eh + 1) * 512], in_=ws_psum[eh])
            # transpose ws -> wst chunks [128, BL] (tiny, K=BL transpose mode)
            wst_sb = []
            id32r = const_pool.tile([BL, BL], f32r)
            nc.vector.tensor_copy(out=id32r, in_=id32[0:BL, 0:BL])
            for dcn in range(DC):
                tp = wh_ps.tile([128, ST], f32r, tag="wh", name="tp_ws")
                nc.tensor.transpose(
                    tp[:, 0:BL], ws_sb[0:BL, dcn * 128:(dcn + 1) * 128], id32r
                )
                w = const_pool.tile([128, BL], f32, tag="wst_sb", bufs=DC, name=f"wst{dcn}")
                nc.vector.tensor_copy(out=w, in_=tp[:, 0:BL])
                wst_sb.append(w)

            # ---- main loop over (batch, s-tile) ----
            # The (exp-columnize + ctx) block of tile t is emitted after tile
            # t+1's WH/score work: the PE would otherwise idle ~1us per tile
            # waiting for ACT's exp. `pending` carries tile t's closure.
            state = {}   # per-b: exp_all, zparts, ctx
            pending = [] # [(bj, st, subs)]

            def emit_ctx(bj, st, subs):
                s0 = st * ST
                exp_all = state[bj]["exp_all"]
                ex = ex_ps.tile([128, NSUB], f32, tag="wh", name="ex")
                for j in range(NSUB):
                    nc.tensor.matmul(
                        ex[:, j:j + 1],
                        exp_all[0:32, s0 + j * 128:s0 + (j + 1) * 128],
                        e0f16,
                        start=True, stop=True,
                    )
                ext = row_pool.tile([128, NSUB], f32r, tag="ext", bufs=3)
                nc.vector.tensor_copy(out=ext, in_=ex)
                # per-tile psum group, folded into the SBUF accumulator by DVE
                ctx_acc = state[bj]["ctx_acc"]
                for eh in range(2):
                    ctx_t = ctx_ps.tile([1, 512], f32, tag="ctx", name="ctx_t")
                    for j in range(NSUB):
                        nc.tensor.matmul(
                            ctx_t,
                            ext[:, j:j + 1],
                            subs[j][:, eh * 512:(eh + 1) * 512],
                            start=(j == 0), stop=(j == NSUB - 1),
                        )
                    sl = ctx_acc[0:1, eh * 512:(eh + 1) * 512]
                    nc.vector.tensor_add(out=sl, in0=sl, in1=ctx_t)

            def finish_batch(bj):
                z = row_pool.tile([1, 1], f32, tag="z")
                nc.vector.reduce_sum(out=z, in_=state[bj]["zparts"], axis=X)
                rz = row_pool.tile([1, 1], f32, tag="rz")
                nc.vector.reciprocal(out=rz, in_=z)
                ctx_sb = row_pool.tile([1, E2], f32, tag="ctx_sb")
                nc.vector.tensor_scalar_mul(
                    out=ctx_sb, in0=state[bj]["ctx_acc"], scalar1=rz,
                )
                nc.sync.dma_start(out=outp[bj:bj + 1, :], in_=ctx_sb)

            for bj in range(BL):
                # [32, S] so the row->column move can be a K=32 matmul against
                # e0 (rows 1-31 are zero); only row 0 holds exp scores.
                # fp16 is safe here: scores are bounded well below fp16's
                # exp-overflow point (|score| <= ~5 for randn-scale inputs,
                # overflow needs >11), and fp16 rounding ~5e-4 matches the
                # TF32 precision used everywhere else.
                exp_all = row_pool.tile([32, S], f16, tag="exp_all")
                nc.vector.memset(exp_all, 0.0)
                zparts = row_pool.tile([1, NST], f32, tag="zparts")
                ctx_acc = row_pool.tile([1, E2], f32, tag="ctx_acc")
                nc.vector.memset(ctx_acc, 0.0)
                state[bj] = dict(exp_all=exp_all, zparts=zparts, ctx_acc=ctx_acc)

                for st in range(NST):
                    s0 = st * ST
                    subs = subs_cache.pop((bj, st), None) or load_subs(bj, st)
                    enct = enct_cache.pop((bj, st), None) or load_enct(bj, st)

                    # WH^T + tanh + score, d-chunks in pairs
                    sc = sc_ps.tile([1, ST], f32, tag="sc")
                    for dp in range(DC // 2):
                        whs = [wh_ps.tile([128, ST], f32, tag="wh", name=f"wh{dd}")
                               for dd in range(2)]
                        for ecn in range(EC):
                            for dd in range(2):
                                dcn = dp * 2 + dd
                                nc.tensor.matmul(
                                    whs[dd],
                                    wct_sb[ecn][:, dcn * 128:(dcn + 1) * 128],
                                    enct[ecn],
                                    start=(ecn == 0), stop=(ecn == EC - 1),
                                )
                        for dd in range(2):
                            dcn = dp * 2 + dd
                            th = tanh_pool.tile([128, ST], f32r, tag="th", name="th")
                            nc.scalar.activation(
                                out=th, in_=whs[dd], func=TANH,
                                bias=wst_sb[dcn][:, bj:bj + 1], scale=1.0,
                            )
                            nc.tensor.matmul(
                                sc, wa_sb[:, dcn:dcn + 1], th,
                                start=(dcn == 0), stop=(dcn == DC - 1),
                            )

                    # exp (+ per-tile partial of Z via accum_out)
                    nc.scalar.activation(
                        out=exp_all[0:1, s0:s0 + ST], in_=sc, func=EXP,
                        accum_out=zparts[0:1, st:st + 1],
                    )

                    # deferred ctx of the previous tile
                    if pending:
                        emit_ctx(*pending.pop())
                    pending.append((bj, st, subs))

                    if st == NST - 1 and bj > 0:
                        # previous batch is fully accumulated once its last
                        # pending ctx ran (one tile ago) -> normalize + store
                        finish_batch(bj - 1)

            emit_ctx(*pending.pop())
            finish_batch(BL - 1)

    nc.finalize()
    return nc


def _prep_inputs(dec_prev_hidden, enc_outputs, Wb, Wc, Wa):
    dec_prev_hidden = np.ascontiguousarray(np.asarray(dec_prev_hidden, dtype=np.float32))
    enc_outputs = np.ascontiguousarray(np.asarray(enc_outputs, dtype=np.float32))
    Wb = np.asarray(Wb, dtype=np.float32)
    Wc = np.asarray(Wc, dtype=np.float32)
    Wa = np.asarray(Wa, dtype=np.float32)

    wbt = np.ascontiguousarray(Wb.T)                     # [d, d2]
    wct = np.ascontiguousarray(Wc.T)                     # [e, d]
    dect = np.ascontiguousarray(dec_prev_hidden.T)       # [D, B]
    wa2 = np.ascontiguousarray(Wa.reshape(DC, 128).T)    # [128, DC]

    in_maps = []
    for i in range(NCORES):
        bsl = slice(i * BL, (i + 1) * BL)
        shard = enc_outputs[:, bsl, :]
        in_maps.append({
            "enc": np.ascontiguousarray(shard),
            "enct": np.ascontiguousarray(shard.transpose(2, 1, 0)),  # [E2, BL, S]
            "dect": np.ascontiguousarray(dect[:, bsl]),
            "wbt": wbt,
            "wct": wct,
            "wa2": wa2,
        })
    return in_maps


def _run(inputs, trace=False):
    from concourse.bass_utils import run_bass_kernel_spmd

    if "nc" not in _CACHE:
        _CACHE["nc"] = _build_nc()
    nc = _CACHE["nc"]
    in_maps = _prep_inputs(**inputs)
    res = run_bass_kernel_spmd(nc, in_maps, list(range(NCORES)), trace=trace)
    out = np.concatenate([res.results[i]["out"] for i in range(NCORES)], axis=0)
    return out[None, :, :].astype(np.float32), res


def kernel(dec_prev_hidden, enc_outputs, Wb, Wc, Wa):
    out, _ = _run(dict(
        dec_prev_hidden=dec_prev_hidden, enc_outputs=enc_outputs,
        Wb=Wb, Wc=Wc, Wa=Wa,
    ))
    return out



# revision 16
# speedup vs baseline: 1.2699x; 1.2699x over previous
"""Bahdanau attention on 8 Trainium2 NeuronCores (Bass/Tile).

Problem:  S=2048, B=32, D=1024, E2=1024
  ws  = dec @ Wb.T                       [B, D]
  WH  = enc @ Wc.T                       [S, B, D]
  sc  = tanh(WH + ws) . Wa               [S, B]
  at  = softmax(sc, axis=0)              [S, B]
  out = einsum('sb,sbe->be', at, enc)[None]   [1, B, 2E]

Sharding: data-parallel over batch B across 8 cores (4 batches/core);
weights replicated. Softmax axis (S) stays core-local.

Dataflow (v2, transposed WH):
  WH is computed TRANSPOSED per s-tile: psum [128s, 512d] with the enc^T
  chunk as the stationary operand and Wc^T as the moving operand. With s on
  partitions and d on the free axis, the whole score reduction moves off
  the PE: ws is added as a free-axis row broadcast (DVE), tanh on ACT, and
  score[s] = sum_d Wa_d tanh(...) is a fused multiply+reduce on DVE
  (tensor_tensor_reduce with chained accumulate across the two d-halves).
  exp then lands directly in column form [128s, 1] (ACT, with accum_out
  providing softmax-Z partials), which feeds the context matmuls as lhsT
  with no transpose/columnize step. The context contraction accumulates
  unnormalized in psum over the four 128-s blocks, folded into an SBUF
  accumulator by DVE, scaled once by 1/Z per batch.

Precision: everything bf16 except psum/accumulators (f32). The first NF8
e-dims of the WH contraction run as fp8e4 (e4m3) pairs with
MatmulPerfMode.DoubleRow, which processes K=256 per matmul at the same
per-matmul cost as K=128 bf16 (HW-measured 259 ns/MM for both at N=512,
LDWEIGHTS fully hidden). exp needs no max-subtraction: |score| <= ~26 so
exp fits fp32/bf16 range comfortably and softmax is shift-invariant.

The PE is the bottleneck (>90% busy); instruction emission order doubles
as the schedule: each tile's context matmuls are emitted after the NEXT
tile's WH matmuls so the PE never waits on the ACT exp; ~20 warmup
matmuls on garbage data ramp the PE p-state during the startup DMAs.
"""

import numpy as np
import ml_dtypes

S, B, D, E2 = 2048, 32, 1024, 1024
NCORES = 8
BL = B // NCORES          # batches per core
ST = 512                  # s-tile size
NST = S // ST             # s-tiles per batch
NSB = ST // 128           # 128-row s-blocks per s-tile
DC = D // 128             # d chunks (ws path)

import os
NF8 = int(os.environ.get("K_NF8", "512"))  # leading e-dims in fp8 DoubleRow
NP8 = NF8 // 256          # fp8 k-pairs
EC16 = (E2 - NF8) // 128  # bf16 e-chunks
_NO_TTR = bool(int(os.environ.get("K_NO_TTR", "1")))  # ttr crashes HW runtime
_NO_WARM = bool(int(os.environ.get("K_NO_WARM", "0")))
_NO_RT = bool(int(os.environ.get("K_NO_RT", "0")))    # skip DRAM ws round-trip
_LIMIT = int(os.environ.get("K_LIMIT", "99"))         # max tiles emitted

_CACHE = {}


def _build_nc():
    import concourse.bacc as bacc
    import concourse.tile as tile
    from concourse import mybir

    f32 = mybir.dt.float32
    bf16 = mybir.dt.bfloat16
    f8 = mybir.dt.float8e4
    TANH = mybir.ActivationFunctionType.Tanh
    EXP = mybir.ActivationFunctionType.Exp
    X = mybir.AxisListType.X
    MUL = mybir.AluOpType.mult
    ADD = mybir.AluOpType.add
    DR = mybir.MatmulPerfMode.DoubleRow

    nc = bacc.Bacc()
    # host-prepped layouts (see _prep_inputs)
    if NP8:
        enct8_h = nc.declare_dram_parameter("enct8", [128, NP8, 2, BL, S], f8, isOutput=False)
        wct8_h = nc.declare_dram_parameter("wct8", [128, NP8, 2, D], f8, isOutput=False)
    if EC16:
        enct16_h = nc.declare_dram_parameter("enct16", [128, EC16, BL, S], bf16, isOutput=False)
        wct16_h = nc.declare_dram_parameter("wct16", [128, EC16, D], bf16, isOutput=False)
    encn_h = nc.declare_dram_parameter("encn", [128, BL, S // 128, E2], bf16, isOutput=False)
    dect_h = nc.declare_dram_parameter("dect", [128, DC, BL], bf16, isOutput=False)
    wbt_h = nc.declare_dram_parameter("wbt", [128, DC, D], bf16, isOutput=False)
    wab_h = nc.declare_dram_parameter("wab", [128, D], bf16, isOutput=False)
    outp = nc.declare_dram_parameter("out", [BL, E2], f32, isOutput=True)
    wsx_h = nc.declare_dram_parameter("wsx", [BL, D], bf16, isOutput=True)

    with tile.TileContext(nc) as tc:
        with (
            tc.tile_pool(name="const", bufs=1) as cp,
            tc.tile_pool(name="e8p", bufs=3) as e8p,
            tc.tile_pool(name="e16p", bufs=3) as e16p,
            tc.tile_pool(name="encn", bufs=3) as enp,
            tc.tile_pool(name="work", bufs=2) as wp,
            tc.tile_pool(name="rows", bufs=2) as rp,
            tc.tile_pool(name="wh_ps", bufs=4, space="PSUM") as wh_ps,
            tc.tile_pool(name="ctx_ps", bufs=2, space="PSUM") as ctx_ps,
        ):
            # ---- tiny consts (DVE, before any PE work) ----
            warm = cp.tile([128, 512], bf16)
            nc.vector.memset(warm, 0.125)
            ones32 = cp.tile([32, 128], bf16)
            nc.vector.memset(ones32, 1.0)
            ones128 = cp.tile([128, 1], f32)
            nc.vector.memset(ones128, 1.0)
            scratch = cp.tile([1, 8], f32)

            # ---- startup DMAs ----
            # gpsimd queue: ws-path inputs first (chunk-paced wbt), then the
            # WH weights; sync queue in parallel: tile(0,0) enc loads + wab.
            dect_sb = cp.tile([128, DC, BL], bf16)
            nc.gpsimd.dma_start(out=dect_sb, in_=dect_h[:, :, :])
            wbt_sb = cp.tile([128, DC, D], bf16)
            for dk in range(DC):
                nc.gpsimd.dma_start(out=wbt_sb[:, dk, :], in_=wbt_h[:, dk, :])

            def load_tile(bj, st, pfx=""):
                s0 = st * ST
                tiles = {}
                if NP8:
                    t8 = e8p.tile([128, NP8, 2, ST], f8, tag="e8", name=f"e8{pfx}")
                    for ep in range(NP8):
                        nc.gpsimd.dma_start(
                            out=t8[:, ep, :, :], in_=enct8_h[:, ep, :, bj, s0:s0 + ST])
                    tiles["t8"] = t8
                if EC16:
                    t16 = e16p.tile([128, EC16, ST], bf16, tag="e16", name=f"e16{pfx}")
                    nc.gpsimd.dma_start(out=t16, in_=enct16_h[:, :, bj, s0:s0 + ST])
                    tiles["t16"] = t16
                tn = enp.tile([128, NSB, E2], bf16, tag="en", name=f"en{pfx}")
                nc.gpsimd.dma_start(out=tn, in_=encn_h[:, bj, st * NSB:(st + 1) * NSB, :])
                tiles["tn"] = tn
                return tiles

            if NP8:
                wct8_sb = cp.tile([128, NP8, 2, D], f8)
                nc.gpsimd.dma_start(out=wct8_sb, in_=wct8_h[:, :, :, :])
            if EC16:
                wct16_sb = cp.tile([128, EC16, D], bf16)
                for c in range(EC16):
                    nc.gpsimd.dma_start(out=wct16_sb[:, c, :], in_=wct16_h[:, c, :])
            tile_cache = {(0, 0): load_tile(0, 0, "00")}
            wab_sb = cp.tile([128, D], bf16)
            nc.sync.dma_start(out=wab_sb, in_=wab_h[:, :])

            # ---- PE warmup: ramp the p-state during startup DMA wait ----
            if not _NO_WARM:
                wps = wh_ps.tile([128, 512], f32, tag="wh", name="warmps")
                for _ in range(20):
                    nc.tensor.matmul(wps, warm[:, 0:128], warm, start=True, stop=True)
                nc.scalar.copy(out=scratch[0:1, 0:4], in_=wps[0:1, 0:4])

            # ---- ws = dec @ Wb.T  [BL, D], then broadcast to [128, D]/batch ----
            ws_ps = [wh_ps.tile([128, 512], f32, tag="wh", name=f"wsps{h}")
                     for h in range(2)]
            for dk in range(DC):
                for h in range(2):
                    nc.tensor.matmul(
                        ws_ps[h][0:BL, :], dect_sb[:, dk, :],
                        wbt_sb[:, dk, h * 512:(h + 1) * 512],
                        start=(dk == 0), stop=(dk == DC - 1),
                    )
            ws_sb = cp.tile([BL, D], bf16)
            for h in range(2):
                nc.scalar.copy(out=ws_sb[:, h * 512:(h + 1) * 512], in_=ws_ps[h][0:BL, :])
            # matmul moving operands must start at partition 0: bounce each
            # batch's ws row through DRAM to partition 0 (padded to K=32 with
            # zeros), then replicate across partitions with a ones matmul.
            nc.sync.dma_start(out=wsx_h[:, :], in_=ws_sb)
            wsb = []
            rows = []
            for bj in range(BL):
                r = cp.tile([32, D], bf16, tag="wsrow", bufs=BL, name=f"wsrow{bj}")
                nc.vector.memset(r, 0.0)
                if _NO_RT:
                    nc.vector.tensor_copy(out=r[0:1, :], in_=ws_sb[0:1, :])
                else:
                    nc.sync.dma_start(out=r[0:1, :], in_=wsx_h[bj:bj + 1, :])
                rows.append(r)
            for bj in range(BL):
                w = cp.tile([128, D], f32, tag="wsb", bufs=BL, name=f"wsb{bj}")
                for h in range(2):
                    bc = wh_ps.tile([128, 512], f32, tag="wh", name=f"bc{bj}{h}")
                    nc.tensor.matmul(
                        bc, ones32, rows[bj][:, h * 512:(h + 1) * 512],
                        start=True, stop=True,
                    )
                    nc.vector.tensor_copy(out=w[:, h * 512:(h + 1) * 512], in_=bc)
                wsb.append(w)

            # ---- main loop ----
            state = {}
            pending = []

            def emit_ctx(bj, st, tn, exp_all, ctx_acc):
                for eh in range(2):
                    ct = ctx_ps.tile([1, 512], f32, tag="ctx", name="ct")
                    for j in range(NSB):
                        nc.tensor.matmul(
                            ct, exp_all[:, st * NSB + j:st * NSB + j + 1],
                            tn[:, j, eh * 512:(eh + 1) * 512],
                            start=(j == 0), stop=(j == NSB - 1),
                        )
                    sl = ctx_acc[0:1, eh * 512:(eh + 1) * 512]
                    nc.vector.tensor_add(out=sl, in0=sl, in1=ct)

            def finish_batch(bj):
                st_ = state[bj]
                zcol = rp.tile([128, 1], f32, tag="zcol")
                nc.vector.reduce_sum(out=zcol, in_=st_["zp4"], axis=X)
                zps = ctx_ps.tile([1, 512], f32, tag="ctx", name="zps")
                nc.tensor.matmul(zps[0:1, 0:1], zcol, ones128, start=True, stop=True)
                rz = rp.tile([1, 1], f32, tag="rz")
                nc.vector.reciprocal(out=rz, in_=zps[0:1, 0:1])
                ctx_out = rp.tile([1, E2], f32, tag="cout")
                nc.vector.tensor_scalar_mul(out=ctx_out, in0=st_["ctx_acc"], scalar1=rz)
                nc.sync.dma_start(out=outp[bj:bj + 1, :], in_=ctx_out)

            tidx = 0
            for bj in range(BL):
                exp_all = rp.tile([128, NST * NSB], bf16, tag="exp")
                zp4 = rp.tile([128, NST], f32, tag="zp4")
                ctx_acc = rp.tile([1, E2], f32, tag="ctxa")
                nc.vector.memset(ctx_acc, 0.0)
                state[bj] = dict(exp_all=exp_all, zp4=zp4, ctx_acc=ctx_acc)

                for st in range(NST):
                    if tidx >= _LIMIT:
                        continue
                    tidx += 1
                    tiles = tile_cache.pop((bj, st), None) or load_tile(bj, st)
                    t8, t16, tn = tiles.get("t8"), tiles.get("t16"), tiles["tn"]

                    scd = rp.tile([128, NSB], f32, tag="scd")
                    sc4 = rp.tile([128, NSB], f32, tag="sc4")
                    for sb in range(NSB):
                        tw = wp.tile([128, D], bf16, tag="tw", bufs=3)
                        for dh in range(2):
                            wh = wh_ps.tile([128, 512], f32, tag="wh", name="wh")
                            k, nk = 0, NP8 + EC16
                            for ep in range(NP8):
                                nc.tensor.matmul(
                                    wh, t8[:, ep, :, sb * 128:(sb + 1) * 128],
                                    wct8_sb[:, ep, :, dh * 512:(dh + 1) * 512],
                                    start=(k == 0), stop=(k == nk - 1), perf_mode=DR,
                                )
                                k += 1
                            for c in range(EC16):
                                nc.tensor.matmul(
                                    wh, t16[:, c, sb * 128:(sb + 1) * 128],
                                    wct16_sb[:, c, dh * 512:(dh + 1) * 512],
                                    start=(k == 0), stop=(k == nk - 1),
                                )
                                k += 1
                            x = wp.tile([128, 512], f32, tag="x", bufs=4)
                            nc.vector.tensor_add(out=x, in0=wh, in1=wsb[bj][:, dh * 512:(dh + 1) * 512])
                            th = tw[:, dh * 512:(dh + 1) * 512]
                            nc.scalar.activation(out=th, in_=x, func=TANH)
                            # score partial: accum = sum_d th*wa (+ prev half)
                            if _NO_TTR:
                                nc.vector.tensor_mul(
                                    out=th, in0=th,
                                    in1=wab_sb[:, dh * 512:(dh + 1) * 512])
                                nc.vector.reduce_sum(
                                    out=(scd if dh == 0 else sc4)[:, sb:sb + 1],
                                    in_=th, axis=X)
                                if dh == 1:
                                    nc.vector.tensor_add(
                                        out=sc4[:, sb:sb + 1], in0=sc4[:, sb:sb + 1],
                                        in1=scd[:, sb:sb + 1])
                            else:
                                nc.vector.tensor_tensor_reduce(
                                    out=th, in0=th, in1=wab_sb[:, dh * 512:(dh + 1) * 512],
                                    scale=1.0,
                                    scalar=0.0 if dh == 0 else scd[:, sb:sb + 1],
                                    op0=MUL, op1=ADD,
                                    accum_out=(scd if dh == 0 else sc4)[:, sb:sb + 1],
                                )
                    nc.scalar.activation(
                        out=exp_all[:, st * NSB:(st + 1) * NSB], in_=sc4, func=EXP,
                        accum_out=zp4[:, st:st + 1],
                    )

                    if pending:
                        emit_ctx(*pending.pop())
                    pending.append((bj, st, tn, exp_all, ctx_acc))

                    if st == NST - 1 and bj > 0 and bj * NST <= _LIMIT:
                        finish_batch(bj - 1)

            if pending:
                emit_ctx(*pending.pop())
            if BL * NST <= _LIMIT:
                finish_batch(BL - 1)

    nc.finalize()
    return nc


def _prep_inputs(dec_prev_hidden, enc_outputs, Wb, Wc, Wa):
    bf = ml_dtypes.bfloat16
    f8 = ml_dtypes.float8_e4m3
    dec = np.asarray(dec_prev_hidden, dtype=np.float32)
    enc = np.asarray(enc_outputs, dtype=np.float32)
    Wb = np.asarray(Wb, dtype=np.float32)
    Wc = np.asarray(Wc, dtype=np.float32)
    Wa = np.asarray(Wa, dtype=np.float32)

    # weights, replicated
    wct = Wc.T                                           # [e, d]
    if NP8:
        wct8 = np.ascontiguousarray(
            wct[:NF8].reshape(NP8, 2, 128, D).transpose(2, 0, 1, 3)).astype(f8)
    if EC16:
        wct16 = np.ascontiguousarray(
            wct[NF8:].reshape(EC16, 128, D).transpose(1, 0, 2)).astype(bf)
    wbt = np.ascontiguousarray(
        Wb.T.reshape(DC, 128, D).transpose(1, 0, 2)).astype(bf)   # [p, c, d2]
    dect = np.ascontiguousarray(
        dec.T.reshape(DC, 128, B).transpose(1, 0, 2)).astype(bf)  # [p, c, b]
    wab = np.ascontiguousarray(np.broadcast_to(Wa[None, :], (128, D))).astype(bf)

    # enc layouts
    enct = enc.transpose(2, 1, 0)                        # [e, b, s]
    if NP8:
        enct8 = np.ascontiguousarray(
            enct[:NF8].reshape(NP8, 2, 128, B, S).transpose(2, 0, 1, 3, 4)).astype(f8)
    if EC16:
        enct16 = np.ascontiguousarray(
            enct[NF8:].reshape(EC16, 128, B, S).transpose(1, 0, 2, 3)).astype(bf)
    # natural: [p, b, jblock, e] with s = jblock*128 + p
    encn = np.ascontiguousarray(
        enc.reshape(S // 128, 128, B, E2).transpose(1, 2, 0, 3)).astype(bf)

    in_maps = []
    for i in range(NCORES):
        bsl = slice(i * BL, (i + 1) * BL)
        m = {
            "encn": np.ascontiguousarray(encn[:, bsl]),
            "dect": np.ascontiguousarray(dect[:, :, bsl]),
            "wbt": wbt,
            "wab": wab,
        }
        if NP8:
            m["enct8"] = np.ascontiguousarray(enct8[:, :, :, bsl])
            m["wct8"] = wct8
        if EC16:
            m["enct16"] = np.ascontiguousarray(enct16[:, :, bsl])
            m["wct16"] = wct16
        in_maps.append(m)
    return in_maps


def _run(inputs, trace=False):
    from concourse.bass_utils import run_bass_kernel_spmd

    if "nc" not in _CACHE:
        _CACHE["nc"] = _build_nc()
    nc = _CACHE["nc"]
    in_maps = _prep_inputs(**inputs)
    res = run_bass_kernel_spmd(nc, in_maps, list(range(NCORES)), trace=trace)
    out = np.concatenate([res.results[i]["out"] for i in range(NCORES)], axis=0)
    return out[None, :, :].astype(np.float32), res


def kernel(dec_prev_hidden, enc_outputs, Wb, Wc, Wa):
    out, _ = _run(dict(
        dec_prev_hidden=dec_prev_hidden, enc_outputs=enc_outputs,
        Wb=Wb, Wc=Wc, Wa=Wa,
    ))
    return out


# revision 25
# speedup vs baseline: 1.3279x; 1.0457x over previous
"""Bahdanau attention on 8 Trainium2 NeuronCores (Bass/Tile).

Problem:  S=2048, B=32, D=1024, E2=1024
  ws  = dec @ Wb.T                       [B, D]
  WH  = enc @ Wc.T                       [S, B, D]
  sc  = tanh(WH + ws) . Wa               [S, B]
  at  = softmax(sc, axis=0)              [S, B]
  out = einsum('sb,sbe->be', at, enc)[None]   [1, B, 2E]

Sharding: data-parallel over batch B across 8 cores (4 batches/core);
weights replicated. Softmax axis (S) stays core-local.

Dataflow (v2, transposed WH):
  WH is computed TRANSPOSED per s-tile: psum [128s, 512d] with the enc^T
  chunk as the stationary operand and Wc^T as the moving operand. With s on
  partitions and d on the free axis, the whole score reduction moves off
  the PE: ws is added as a free-axis row broadcast (DVE), tanh on ACT, and
  score[s] = sum_d Wa_d tanh(...) is a fused multiply+reduce on DVE
  (tensor_tensor_reduce with chained accumulate across the two d-halves).
  exp then lands directly in column form [128s, 1] (ACT, with accum_out
  providing softmax-Z partials), which feeds the context matmuls as lhsT
  with no transpose/columnize step. The context contraction accumulates
  unnormalized in psum over the four 128-s blocks, folded into an SBUF
  accumulator by DVE, scaled once by 1/Z per batch.

Precision: everything bf16 except psum/accumulators (f32). The first NF8
e-dims of the WH contraction run as fp8e4 (e4m3) pairs with
MatmulPerfMode.DoubleRow, which processes K=256 per matmul at the same
per-matmul cost as K=128 bf16 (HW-measured 259 ns/MM for both at N=512,
LDWEIGHTS fully hidden). exp needs no max-subtraction: |score| <= ~26 so
exp fits fp32/bf16 range comfortably and softmax is shift-invariant.

The PE is the bottleneck (>90% busy); instruction emission order doubles
as the schedule: each tile's context matmuls are emitted after the NEXT
tile's WH matmuls so the PE never waits on the ACT exp; ~20 warmup
matmuls on garbage data ramp the PE p-state during the startup DMAs.
"""

import numpy as np
import ml_dtypes

S, B, D, E2 = 2048, 32, 1024, 1024
NCORES = 8
BL = B // NCORES          # batches per core
ST = 512                  # s-tile size
NST = S // ST             # s-tiles per batch
NSB = ST // 128           # 128-row s-blocks per s-tile
DC = D // 128             # d chunks (ws path)

import os
NF8 = int(os.environ.get("K_NF8", "512"))  # leading e-dims in fp8 DoubleRow
NP8 = NF8 // 256          # fp8 k-pairs
EC16 = (E2 - NF8) // 128  # bf16 e-chunks
_NO_TTR = bool(int(os.environ.get("K_NO_TTR", "1")))  # ttr crashes HW runtime
_NO_WARM = bool(int(os.environ.get("K_NO_WARM", "0")))
_NO_RT = bool(int(os.environ.get("K_NO_RT", "0")))    # skip DRAM ws round-trip
_LIMIT = int(os.environ.get("K_LIMIT", "99"))         # max tiles emitted
_AMR = bool(int(os.environ.get("K_AMR", "1")))        # fused affine_mul_reduce
_GPADD = bool(int(os.environ.get("K_GPADD", "0")))    # ws-add on gpsimd
_WSMM = bool(int(os.environ.get("K_WSMM", "1")))      # fold ws into psum via PE

_CACHE = {}


def _build_nc():
    import concourse.bacc as bacc
    import concourse.tile as tile
    from concourse import mybir

    f32 = mybir.dt.float32
    bf16 = mybir.dt.bfloat16
    f8 = mybir.dt.float8e4
    TANH = mybir.ActivationFunctionType.Tanh
    EXP = mybir.ActivationFunctionType.Exp
    X = mybir.AxisListType.X
    MUL = mybir.AluOpType.mult
    ADD = mybir.AluOpType.add
    DR = mybir.MatmulPerfMode.DoubleRow

    nc = bacc.Bacc()
    # host-prepped layouts (see _prep_inputs)
    if NP8:
        enct8_h = nc.declare_dram_parameter("enct8", [128, NP8, 2, BL, S], f8, isOutput=False)
        wct8_h = nc.declare_dram_parameter("wct8", [128, NP8, 2, D], f8, isOutput=False)
    if EC16:
        enct16_h = nc.declare_dram_parameter("enct16", [128, EC16, BL, S], bf16, isOutput=False)
        wct16_h = nc.declare_dram_parameter("wct16", [128, EC16, D], bf16, isOutput=False)
    encn_h = nc.declare_dram_parameter("encn", [128, BL, S // 128, E2], bf16, isOutput=False)
    dect_h = nc.declare_dram_parameter("dect", [128, DC, BL], bf16, isOutput=False)
    wbt_h = nc.declare_dram_parameter("wbt", [128, DC, D], bf16, isOutput=False)
    wab_h = nc.declare_dram_parameter("wab", [128, D], bf16, isOutput=False)
    outp = nc.declare_dram_parameter("out", [BL, E2], f32, isOutput=True)
    wsx_h = nc.declare_dram_parameter("wsx", [BL, D], bf16, isOutput=True)

    with tile.TileContext(nc) as tc:
        with (
            tc.tile_pool(name="const", bufs=1) as cp,
            tc.tile_pool(name="e8p", bufs=3) as e8p,
            tc.tile_pool(name="e16p", bufs=3) as e16p,
            tc.tile_pool(name="encn", bufs=3) as enp,
            tc.tile_pool(name="work", bufs=2) as wp,
            tc.tile_pool(name="rows", bufs=2) as rp,
            tc.tile_pool(name="wh_ps", bufs=4, space="PSUM") as wh_ps,
            tc.tile_pool(name="ctx_ps", bufs=2, space="PSUM") as ctx_ps,
        ):
            # ---- tiny consts (DVE, before any PE work) ----
            warm = cp.tile([128, 512], bf16)
            nc.vector.memset(warm, 0.125)
            ones32 = cp.tile([32, 128], bf16)
            nc.vector.memset(ones32, 1.0)
            ones128 = cp.tile([128, 1], f32)
            nc.vector.memset(ones128, 1.0)
            scratch = cp.tile([1, 8], f32)
            # ws rows (partition 0, K=32 padded): memset the zero rows now so
            # only the row-0 DMA is on the ws critical path later.
            rows = []
            for bj in range(BL):
                r = cp.tile([32, D], bf16, tag="wsrow", bufs=BL, name=f"wsrow{bj}")
                nc.vector.memset(r, 0.0)
                rows.append(r)

            # ---- startup DMAs ----
            # gpsimd queue: ws-path inputs first (chunk-paced wbt), then the
            # WH weights; sync queue in parallel: tile(0,0) enc loads + wab.
            dect_sb = cp.tile([128, DC, BL], bf16)
            nc.gpsimd.dma_start(out=dect_sb, in_=dect_h[:, :, :])
            wbt_sb = cp.tile([128, DC, D], bf16)
            for dk in range(DC):
                nc.gpsimd.dma_start(out=wbt_sb[:, dk, :], in_=wbt_h[:, dk, :])

            def load_tile(bj, st, pfx=""):
                s0 = st * ST
                tiles = {}
                if NP8:
                    t8 = e8p.tile([128, NP8, 2, ST], f8, tag="e8", name=f"e8{pfx}")
                    for ep in range(NP8):
                        nc.gpsimd.dma_start(
                            out=t8[:, ep, :, :], in_=enct8_h[:, ep, :, bj, s0:s0 + ST])
                    tiles["t8"] = t8
                if EC16:
                    t16 = e16p.tile([128, EC16, ST], bf16, tag="e16", name=f"e16{pfx}")
                    nc.gpsimd.dma_start(out=t16, in_=enct16_h[:, :, bj, s0:s0 + ST])
                    tiles["t16"] = t16
                tn = enp.tile([128, NSB, E2], bf16, tag="en", name=f"en{pfx}")
                nc.gpsimd.dma_start(out=tn, in_=encn_h[:, bj, st * NSB:(st + 1) * NSB, :])
                tiles["tn"] = tn
                return tiles

            if NP8:
                wct8_sb = cp.tile([128, NP8, 2, D], f8)
                nc.gpsimd.dma_start(out=wct8_sb, in_=wct8_h[:, :, :, :])
            if EC16:
                wct16_sb = cp.tile([128, EC16, D], bf16)
                for c in range(EC16):
                    nc.gpsimd.dma_start(out=wct16_sb[:, c, :], in_=wct16_h[:, c, :])
            tile_cache = {(0, 0): load_tile(0, 0, "00")}
            wab_sb = cp.tile([128, D], bf16)
            nc.sync.dma_start(out=wab_sb, in_=wab_h[:, :])

            # ---- PE warmup: ramp the p-state during startup DMA wait ----
            if not _NO_WARM:
                wps = wh_ps.tile([128, 512], f32, tag="wh", name="warmps")
                for _ in range(12):
                    nc.tensor.matmul(wps, warm[:, 0:128], warm, start=True, stop=True)
                nc.scalar.copy(out=scratch[0:1, 0:4], in_=wps[0:1, 0:4])

            # ---- ws = dec @ Wb.T  [BL, D], then broadcast to [128, D]/batch ----
            ws_ps = [wh_ps.tile([128, 512], f32, tag="wh", name=f"wsps{h}")
                     for h in range(2)]
            for dk in range(DC):
                for h in range(2):
                    nc.tensor.matmul(
                        ws_ps[h][0:BL, :], dect_sb[:, dk, :],
                        wbt_sb[:, dk, h * 512:(h + 1) * 512],
                        start=(dk == 0), stop=(dk == DC - 1),
                    )
            ws_sb = cp.tile([BL, D], bf16)
            for h in range(2):
                nc.scalar.copy(out=ws_sb[:, h * 512:(h + 1) * 512], in_=ws_ps[h][0:BL, :])
            # matmul moving operands must start at partition 0: bounce each
            # batch's ws row through DRAM to partition 0 (padded to K=32 with
            # zeros), then replicate across partitions with a ones matmul.
            nc.sync.dma_start(out=wsx_h[:, :], in_=ws_sb)
            for bj in range(BL):
                if _NO_RT:
                    nc.vector.tensor_copy(out=rows[bj][0:1, :], in_=ws_sb[0:1, :])
                else:
                    nc.sync.dma_start(out=rows[bj][0:1, :], in_=wsx_h[bj:bj + 1, :])
            wsb = []
            if not _WSMM:
                for bj in range(BL):
                    w = cp.tile([128, D], f32, tag="wsb", bufs=BL, name=f"wsb{bj}")
                    for h in range(2):
                        bc = wh_ps.tile([128, 512], f32, tag="wh", name=f"bc{bj}{h}")
                        nc.tensor.matmul(
                            bc, ones32, rows[bj][:, h * 512:(h + 1) * 512],
                            start=True, stop=True,
                        )
                        nc.vector.tensor_copy(out=w[:, h * 512:(h + 1) * 512], in_=bc)
                    wsb.append(w)

            # ---- main loop ----
            state = {}
            pending = []

            def emit_ctx(bj, st, tn, exp_all, ctx_acc):
                for eh in range(2):
                    ct = ctx_ps.tile([1, 512], f32, tag="ctx", name="ct")
                    for j in range(NSB):
                        nc.tensor.matmul(
                            ct, exp_all[:, st * NSB + j:st * NSB + j + 1],
                            tn[:, j, eh * 512:(eh + 1) * 512],
                            start=(j == 0), stop=(j == NSB - 1),
                        )
                    sl = ctx_acc[0:1, eh * 512:(eh + 1) * 512]
                    nc.vector.tensor_add(out=sl, in0=sl, in1=ct)

            def finish_batch(bj):
                st_ = state[bj]
                zcol = rp.tile([128, 1], f32, tag="zcol")
                nc.vector.reduce_sum(out=zcol, in_=st_["exp_all"], axis=X)
                zps = ctx_ps.tile([1, 512], f32, tag="ctx", name="zps")
                nc.tensor.matmul(zps[0:1, 0:1], zcol, ones128, start=True, stop=True)
                rz = rp.tile([1, 1], f32, tag="rz")
                nc.vector.reciprocal(out=rz, in_=zps[0:1, 0:1])
                ctx_out = rp.tile([1, E2], f32, tag="cout")
                nc.vector.tensor_scalar_mul(out=ctx_out, in0=st_["ctx_acc"], scalar1=rz)
                nc.sync.dma_start(out=outp[bj:bj + 1, :], in_=ctx_out)

            tidx = 0
            for bj in range(BL):
                exp_all = rp.tile([128, NST * NSB], bf16, tag="exp")
                ctx_acc = rp.tile([1, E2], f32, tag="ctxa")
                nc.vector.memset(ctx_acc, 0.0)
                state[bj] = dict(exp_all=exp_all, ctx_acc=ctx_acc)

                for st in range(NST):
                    if tidx >= _LIMIT:
                        continue
                    tidx += 1
                    tiles = tile_cache.pop((bj, st), None) or load_tile(bj, st)
                    t8, t16, tn = tiles.get("t8"), tiles.get("t16"), tiles["tn"]
                    last_tile = (bj == BL - 1 and st == NST - 1)
                    cts = None

                    if last_tile and pending:
                        emit_ctx(*pending.pop())

                    for sb in range(NSB):
                        tw = wp.tile([128, D], bf16, tag="tw", bufs=3)
                        scd = rp.tile([128, 1], f32, tag="scd", bufs=3)
                        sc1 = rp.tile([128, 1], f32, tag="sc1", bufs=3)
                        for dh in range(2):
                            wh = wh_ps.tile([128, 512], f32, tag="wh", name="wh")
                            k, nk = 0, NP8 + EC16 + (1 if _WSMM else 0)
                            for ep in range(NP8):
                                nc.tensor.matmul(
                                    wh, t8[:, ep, :, sb * 128:(sb + 1) * 128],
                                    wct8_sb[:, ep, :, dh * 512:(dh + 1) * 512],
                                    start=(k == 0), stop=(k == nk - 1), perf_mode=DR,
                                )
                                k += 1
                            for c in range(EC16):
                                nc.tensor.matmul(
                                    wh, t16[:, c, sb * 128:(sb + 1) * 128],
                                    wct16_sb[:, c, dh * 512:(dh + 1) * 512],
                                    start=(k == 0), stop=(k == nk - 1),
                                )
                                k += 1
                            if _WSMM:
                                # +ws as one more accumulating matmul: rows 1-31
                                # of rows[bj] are zero, row 0 holds ws[bj].
                                nc.tensor.matmul(
                                    wh, ones32, rows[bj][:, dh * 512:(dh + 1) * 512],
                                    start=False, stop=True,
                                )
                                src = wh
                            else:
                                x = wp.tile([128, 512], f32, tag="x", bufs=4)
                                eng = nc.gpsimd if _GPADD else nc.vector
                                eng.tensor_add(out=x, in0=wh, in1=wsb[bj][:, dh * 512:(dh + 1) * 512])
                                src = x
                            th = tw[:, dh * 512:(dh + 1) * 512]
                            nc.scalar.activation(out=th, in_=src, func=TANH)
                            # score partial: sum_d th*wa
                            tgt = scd if dh == 0 else sc1
                            if _AMR:
                                nc.vector.affine_mul_reduce(
                                    out=th, accum_out=tgt, in0=th,
                                    in1=wab_sb[:, dh * 512:(dh + 1) * 512],
                                    scale=1.0, bias=0.0,
                                )
                            else:
                                nc.vector.tensor_mul(
                                    out=th, in0=th,
                                    in1=wab_sb[:, dh * 512:(dh + 1) * 512])
                                nc.vector.reduce_sum(out=tgt, in_=th, axis=X)
                        nc.vector.tensor_add(out=sc1, in0=sc1, in1=scd)
                        nc.scalar.activation(
                            out=exp_all[:, st * NSB + sb:st * NSB + sb + 1],
                            in_=sc1, func=EXP,
                        )
                        if sb == 0 and not last_tile and pending:
                            emit_ctx(*pending.pop())
                        if last_tile:
                            # inline the final tile's context matmuls per
                            # s-block so the tail is one block's latency, not
                            # a whole tile's.
                            if cts is None:
                                cts = [ctx_ps.tile([1, 512], f32, tag="ctx",
                                                   name=f"ctl{eh}")
                                       for eh in range(2)]
                            for eh in range(2):
                                nc.tensor.matmul(
                                    cts[eh],
                                    exp_all[:, st * NSB + sb:st * NSB + sb + 1],
                                    tn[:, sb, eh * 512:(eh + 1) * 512],
                                    start=(sb == 0), stop=(sb == NSB - 1),
                                )

                    if last_tile:
                        for eh in range(2):
                            sl = ctx_acc[0:1, eh * 512:(eh + 1) * 512]
                            nc.vector.tensor_add(out=sl, in0=sl, in1=cts[eh])
                    else:
                        pending.append((bj, st, tn, exp_all, ctx_acc))

                    if st == NST - 1 and bj > 0 and bj * NST <= _LIMIT:
                        finish_batch(bj - 1)

            if pending:
                emit_ctx(*pending.pop())
            if BL * NST <= _LIMIT:
                finish_batch(BL - 1)

    nc.finalize()
    return nc


def _prep_inputs(dec_prev_hidden, enc_outputs, Wb, Wc, Wa):
    bf = ml_dtypes.bfloat16
    f8 = ml_dtypes.float8_e4m3
    dec = np.asarray(dec_prev_hidden, dtype=np.float32)
    enc = np.asarray(enc_outputs, dtype=np.float32)
    Wb = np.asarray(Wb, dtype=np.float32)
    Wc = np.asarray(Wc, dtype=np.float32)
    Wa = np.asarray(Wa, dtype=np.float32)

    # weights, replicated
    wct = Wc.T                                           # [e, d]
    if NP8:
        wct8 = np.ascontiguousarray(
            wct[:NF8].reshape(NP8, 2, 128, D).transpose(2, 0, 1, 3)).astype(f8)
    if EC16:
        wct16 = np.ascontiguousarray(
            wct[NF8:].reshape(EC16, 128, D).transpose(1, 0, 2)).astype(bf)
    wbt = np.ascontiguousarray(
        Wb.T.reshape(DC, 128, D).transpose(1, 0, 2)).astype(bf)   # [p, c, d2]
    dect = np.ascontiguousarray(
        dec.T.reshape(DC, 128, B).transpose(1, 0, 2)).astype(bf)  # [p, c, b]
    wab = np.ascontiguousarray(np.broadcast_to(Wa[None, :], (128, D))).astype(bf)

    # enc layouts
    enct = enc.transpose(2, 1, 0)                        # [e, b, s]
    if NP8:
        enct8 = np.ascontiguousarray(
            enct[:NF8].reshape(NP8, 2, 128, B, S).transpose(2, 0, 1, 3, 4)).astype(f8)
    if EC16:
        enct16 = np.ascontiguousarray(
            enct[NF8:].reshape(EC16, 128, B, S).transpose(1, 0, 2, 3)).astype(bf)
    # natural: [p, b, jblock, e] with s = jblock*128 + p
    encn = np.ascontiguousarray(
        enc.reshape(S // 128, 128, B, E2).transpose(1, 2, 0, 3)).astype(bf)

    in_maps = []
    for i in range(NCORES):
        bsl = slice(i * BL, (i + 1) * BL)
        m = {
            "encn": np.ascontiguousarray(encn[:, bsl]),
            "dect": np.ascontiguousarray(dect[:, :, bsl]),
            "wbt": wbt,
            "wab": wab,
        }
        if NP8:
            m["enct8"] = np.ascontiguousarray(enct8[:, :, :, bsl])
            m["wct8"] = wct8
        if EC16:
            m["enct16"] = np.ascontiguousarray(enct16[:, :, bsl])
            m["wct16"] = wct16
        in_maps.append(m)
    return in_maps


def _run(inputs, trace=False):
    from concourse.bass_utils import run_bass_kernel_spmd

    if "nc" not in _CACHE:
        _CACHE["nc"] = _build_nc()
    nc = _CACHE["nc"]
    in_maps = _prep_inputs(**inputs)
    res = run_bass_kernel_spmd(nc, in_maps, list(range(NCORES)), trace=trace)
    out = np.concatenate([res.results[i]["out"] for i in range(NCORES)], axis=0)
    return out[None, :, :].astype(np.float32), res


def kernel(dec_prev_hidden, enc_outputs, Wb, Wc, Wa):
    out, _ = _run(dict(
        dec_prev_hidden=dec_prev_hidden, enc_outputs=enc_outputs,
        Wb=Wb, Wc=Wc, Wa=Wa,
    ))
    return out
